# revision 29
# baseline (speedup 1.0000x reference)
"""BlackwellLinear Trainium2 kernel: 2:4 sparsity + int8 fake-quant + x @ w.T + bias.

Full inputs in, full output out. Data-parallel over tokens across 8 NeuronCores;
weight/bias replicated. All module math (sparsify, quantize, matmul, bias) runs
on device; the host only re-encodes layouts: x is transposed to [in_f, tokens]
fp16, and the in_features axis of both x.T and w.T is permuted phase-major
(p <-> 4*(p%256) + p//256) so each 2:4 group-of-4 spans four k-tiles at the
same partition/column coordinates -- the sparsify+quantize pipeline is then
contiguous full-width elementwise ops and the quantized weight lands directly
in [in_f, out_f] (lhsT) layout. A contraction-axis permutation applied to both
operands leaves the matmul result unchanged.

Numerics (harness gate: rel_err < 2e-2 on max|err|/max|y|; this kernel lands
~1e-3): q = rne(w * inv) with inv = qmax/absmax via Newton-refined reciprocal
(vs the reference's rne(w/scale): off-by-one rounding on ~1e-5 of weights,
harmless). The 2:4 threshold (2nd-largest |w| per group) and its comparisons
run in fp32 -- fp16 compare ties would occasionally keep 3 weights of a group,
which costs ~1 full weight of error on affected rows. The rne uses the
magic-constant trick (+-1.5*2^23). x is a single fp16 plane (error 2^-11,
~3e-4 on the output envelope); dequant scale and bias fold into the PSUM
eviction, emitted as fp16 and upcast on the host.

Schedule: the vector engine is the binding resource (~34 full-width f32 ops;
TT runs only there on this toolchain). Its FIFO is emitted in dependency-
arrival order: pair-max/min ops interleaved with the weight DMA, the global
absmax reduce as early as possible (gpsimd cross-partition max overlaps the
threshold chain), then mask-compare + quantize-apply pairs in exactly the
order the PE consumes k-tiles (evens first -- PSUM accumulation order is
free). Scalar engine: |w| tiles, the rounding ACT, PSUM evictions. The last
mi block's eviction is split across scalar+vector and two DMA queues to
shorten the tail.
"""

import numpy as np

N_CORES = 8
P = 128
IN_F = 1024
OUT_F = 1024
TOKENS = 32768
TOK_PER_CORE = TOKENS // N_CORES  # 4096
K_TILES = IN_F // P  # 8
M_TILES = OUT_F // P  # 8
MM_N = 512  # moving free dim per matmul (one PSUM bank of fp32)
HALF_TOK = TOK_PER_CORE // 2  # 2048
TJ = HALF_TOK // MM_N  # 4 PSUM banks per (half, mi)

MAGIC = 12582912.0  # 1.5 * 2**23: (v + MAGIC) - MAGIC == RNE round for |v| <= 2**22

KI_ORDER = (0, 2, 4, 6, 1, 3, 5, 7)  # evens first: range-0 prep finishes first

# phase-major permutation of the in_features axis: position p holds original
# feature 4*(p%256) + p//256, so k-tile kt covers phase kt//2 of group range
# (kt%2)*128..+128 and the four phases of a group share partition/column coords
_PERM = (4 * (np.arange(IN_F) % 256) + np.arange(IN_F) // 256).astype(np.int64)

_CACHE = {}


def _build(qmax: float):
    from contextlib import ExitStack

    import concourse.tile as tile
    import concourse.mybir as mybir
    from concourse import bacc, bass_isa

    f32 = mybir.dt.float32
    f16 = mybir.dt.float16
    Alu = mybir.AluOpType
    Act = mybir.ActivationFunctionType

    inv_qmax = float(np.float32(1.0) / np.float32(qmax))
    qmaxf = float(np.float32(qmax))

    nc = bacc.Bacc("TRN2", target_bir_lowering=False, debug=False)
    xth = nc.dram_tensor("xth", [IN_F, TOK_PER_CORE], f16, kind="ExternalInput").ap()
    # wp: w.T with permuted in_f rows = [in_f_perm, out_f], fp32
    wp = nc.dram_tensor("wp", [IN_F, OUT_F], f32, kind="ExternalInput").ap()
    # bias pre-tiled on host to [128, 8] (btile[p, mi] = bias[mi*128+p]) so it
    # loads as ONE contiguous DMA instead of 8 small strided column loads
    biast = nc.dram_tensor("biast", [P, M_TILES], f32, kind="ExternalInput").ap()
    yt = nc.dram_tensor("yt", [OUT_F, TOK_PER_CORE], f16, kind="ExternalOutput").ap()

    with tile.TileContext(nc) as tc, ExitStack() as ctx:
        const = ctx.enter_context(tc.tile_pool(name="const", bufs=1))
        wnat_p = ctx.enter_context(tc.tile_pool(name="wnat", bufs=8))
        abs_p = ctx.enter_context(tc.tile_pool(name="absp", bufs=8))
        tmp_p = ctx.enter_context(tc.tile_pool(name="tmp", bufs=8))
        gm_p = ctx.enter_context(tc.tile_pool(name="gm", bufs=1))
        thr_p = ctx.enter_context(tc.tile_pool(name="thr", bufs=2))
        m_p = ctx.enter_context(tc.tile_pool(name="mask", bufs=3))
        q0_p = ctx.enter_context(tc.tile_pool(name="q0", bufs=2))
        qr_p = ctx.enter_context(tc.tile_pool(name="qr", bufs=1))
        wqt_p = ctx.enter_context(tc.tile_pool(name="wqt", bufs=8))
        sc_p = ctx.enter_context(tc.tile_pool(name="sc", bufs=1))
        x_p = ctx.enter_context(tc.tile_pool(name="x", bufs=8))
        y_p = ctx.enter_context(tc.tile_pool(name="y", bufs=2))
        psum_mm = ctx.enter_context(tc.tile_pool(name="psmm", bufs=8, space="PSUM"))

        # ---- all bulk DMA on the sync ring, in priority order: weights (in
        # pair order for the threshold chains), then x. One ring sustains
        # ~730 GB/s here, and a single ring means no arbitration surprises
        # and no compute engine ever blocks on a busy ring (the sync engine
        # has nothing else to do). Tile may reorder same-engine triggers, but
        # everything on this ring is order-insensitive among itself once the
        # weights lead; x consumers run ~25 us after the last x lands. ----
        wk = [None] * K_TILES
        for kt in (0, 2, 4, 6, 1, 3, 5, 7):
            wt = wnat_p.tile([P, OUT_F], f32, tag="wnat", name=f"wnat{kt}")
            nc.sync.dma_start(wt[:], wp[kt * P : (kt + 1) * P, :])
            wk[kt] = wt
        btile = const.tile([P, M_TILES], f32, tag="biast")
        nc.scalar.dma_start(btile[:], biast[:])
        bias_t = [btile[:, mi : mi + 1] for mi in range(M_TILES)]

        # junk PSUM tile for warm-up matmuls: first slot of the "ps" ring;
        # real banks recycle over it long after the dummies retire
        junk_ps = psum_mm.tile([P, MM_N], f32, tag="ps", name="junk")

        # ---- |w| tiles on the scalar engine (pair ops + mask compares) ----
        ak = [None] * K_TILES
        for kt in KI_ORDER:
            a = abs_p.tile([P, OUT_F], f32, tag="abs", name=f"abs{kt}")
            nc.scalar.activation(a[:], wk[kt][:], Act.Abs)
            ak[kt] = a

        # ---- x DMA: behind the weights on the sync ring ----
        xh = [None] * K_TILES
        for ki in KI_ORDER:
            xt = x_p.tile([P, TOK_PER_CORE], f16, tag="xh", name=f"xh{ki}")
            nc.sync.dma_start(xt[:], xth[ki * P : (ki + 1) * P, :])
            xh[ki] = xt

        def vtt(out, in0, in1, op):
            nc.vector.tensor_tensor(out=out, in0=in0, in1=in1, op=op)

        def tmp(name):
            return tmp_p.tile([P, OUT_F], f32, tag="tmp", name=name)

        # ---- vector FIFO, part 1: pair max/min in DMA-arrival order, then
        # global absmax (gm accumulated in place), cross-partition max on
        # gpsimd, threshold chains (2nd-largest = max(min of pair maxes,
        # max of pair mins); in-place accumulation keeps the ring small) ----
        tA0, tB0, tA1, tB1 = tmp("tA0"), tmp("tB0"), tmp("tA1"), tmp("tB1")
        n010, n230, n011, n231 = tmp("n010"), tmp("n230"), tmp("n011"), tmp("n231")
        vtt(tA0[:], ak[0][:], ak[2][:], Alu.max)
        vtt(n010[:], ak[0][:], ak[2][:], Alu.min)
        vtt(tB0[:], ak[4][:], ak[6][:], Alu.max)
        vtt(n230[:], ak[4][:], ak[6][:], Alu.min)
        vtt(tA1[:], ak[1][:], ak[3][:], Alu.max)
        vtt(n011[:], ak[1][:], ak[3][:], Alu.min)
        vtt(tB1[:], ak[5][:], ak[7][:], Alu.max)
        vtt(n231[:], ak[5][:], ak[7][:], Alu.min)
        # own pool: the "tmp" ring is exactly filled by the 8 pair tiles, and
        # a 9th alloc there would recycle tA0 before its later readers exist
        # (bufs is per-tag, so a separate bufs=1 pool costs one buffer)
        gm = gm_p.tile([P, OUT_F], f32, tag="gm", name="gmax")
        vtt(gm[:], tA0[:], tB0[:], Alu.max)
        vtt(gm[:], gm[:], tA1[:], Alu.max)
        vtt(gm[:], gm[:], tB1[:], Alu.max)
        amc = sc_p.tile([P, 1], f32, tag="amc")
        nc.vector.tensor_reduce(
            out=amc[:], in_=gm[:], axis=mybir.AxisListType.X, op=Alu.max
        )
        am = sc_p.tile([P, 1], f32, tag="am")
        nc.gpsimd.partition_all_reduce(
            am[:], amc[:], channels=P, reduce_op=bass_isa.ReduceOp.max
        )
        thr0 = thr_p.tile([P, OUT_F], f32, tag="thr", name="thr0")
        thr1 = thr_p.tile([P, OUT_F], f32, tag="thr", name="thr1")
        vtt(thr0[:], tA0[:], tB0[:], Alu.min)
        vtt(n010[:], n010[:], n230[:], Alu.max)
        vtt(thr0[:], thr0[:], n010[:], Alu.max)

        masks = [None] * K_TILES

        def isge(kt, thr):
            m = m_p.tile([P, OUT_F], f16, tag="mask", name=f"m{kt}")
            vtt(m[:], ak[kt][:], thr[:], Alu.is_ge)
            masks[kt] = m

        isge(0, thr0)
        vtt(thr1[:], tA1[:], tB1[:], Alu.min)
        vtt(n011[:], n011[:], n231[:], Alu.max)
        vtt(thr1[:], thr1[:], n011[:], Alu.max)

        # ---- scale smalls: s = absmax/qmax; inv = qmax * (1 Newton recip) ----
        s_t = sc_p.tile([P, 1], f32, tag="s")
        nc.vector.tensor_scalar(
            out=s_t[:], in0=am[:], scalar1=inv_qmax, scalar2=None, op0=Alu.mult
        )
        r0 = sc_p.tile([P, 1], f32, tag="r0")
        e0 = sc_p.tile([P, 1], f32, tag="e0")
        r1 = sc_p.tile([P, 1], f32, tag="r1")
        inv_t = sc_p.tile([P, 1], f32, tag="inv")
        nc.vector.reciprocal(r0[:], am[:])
        vtt(e0[:], am[:], r0[:], Alu.mult)
        nc.vector.tensor_scalar(
            out=e0[:], in0=e0[:], scalar1=2.0, scalar2=-1.0, op0=Alu.subtract,
            op1=Alu.mult,
        )  # e = 2 - am*r0
        vtt(r1[:], r0[:], e0[:], Alu.mult)
        nc.vector.tensor_scalar(
            out=inv_t[:], in0=r1[:], scalar1=qmaxf, scalar2=None, op0=Alu.mult
        )
        magic_t = sc_p.tile([P, 1], f32, tag="magic")
        nc.gpsimd.memset(magic_t[:], MAGIC)

        # ---- part 2: per k-tile in PE consumption order, interleaving the
        # scalar round-ACT (q0 = w*inv + MAGIC) with the vector mask-compare
        # and quantize-apply (q16 = (q0 - MAGIC) * mask -> fp16). Emission
        # interleave keeps each pool ring's recycle behind its readers.
        # Rounding commutes with the 0/1 mask; clip is a no-op. ----
        wqt = [None] * K_TILES

        def act1(kt):
            q0 = q0_p.tile([P, OUT_F], f32, tag="q0", name=f"q0_{kt}")
            nc.scalar.activation(
                q0[:], wk[kt][:], Act.Identity, bias=magic_t[:], scale=inv_t[:]
            )
            return q0

        def stt(kt, q0):
            q16 = wqt_p.tile([P, OUT_F], f16, tag="q16", name=f"q16_{kt}")
            nc.vector.scalar_tensor_tensor(
                out=q16[:], in0=q0[:], scalar=-MAGIC, in1=masks[kt][:],
                op0=Alu.add, op1=Alu.mult,
            )
            wqt[kt] = q16

        # ---- PE warm-up: the HAM clock gate holds an idle PE at 1.2 GHz and
        # takes ~3.4 us of sustained activity to release. Junk matmuls gated
        # on late prep tiles keep the PE busy right up to the real stream so
        # the real matmuls start at full clock. ----
        for d in range(4):
            nc.tensor.matmul(
                junk_ps[:], masks[0][:, 0:P], masks[0][:, 0:MM_N],
                start=True, stop=True,
            )

        q0_first = act1(0)
        for d in range(5):
            nc.tensor.matmul(
                junk_ps[:, 0:256], q0_first[:, 0:P], q0_first[:, 0:256],
                start=True, stop=True,
            )
        stt(0, q0_first)
        for kt in (2, 4, 6, 1, 3, 5, 7):
            q0 = act1(kt)
            isge(kt, thr0 if kt % 2 == 0 else thr1)
            stt(kt, q0)

        # ---- matmul: yt[m, t] = s * (wqt[k,m].T @ xh[k,t]) + bias[m].
        # mi=0 first runs over the FULL 4096 tokens (all 8 PSUM banks), which
        # stretches the deadline for the late odd k-tiles to ~12 us after PE
        # start; the rest runs token-half x mi x ki with 4+4 bank rotation.
        # The very last group is per-bank ki-sweeps so eviction and store
        # start as each bank completes instead of after the whole group. ----
        def mm_group(mi, tcol0, banks, name, ki_inner=False, split_evict=False):
            ps = [
                psum_mm.tile([P, MM_N], f32, tag="ps", name=f"ps{name}_{tj}")
                for tj in range(banks)
            ]
            if ki_inner:
                for tj in range(banks):
                    lo = tcol0 + tj * MM_N
                    for kn, ki in enumerate(KI_ORDER):
                        nc.tensor.matmul(
                            ps[tj][:],
                            wqt[ki][:, mi * P : (mi + 1) * P],
                            xh[ki][:, lo : lo + MM_N],
                            start=(kn == 0),
                            stop=(kn == K_TILES - 1),
                        )
            else:
                for kn, ki in enumerate(KI_ORDER):
                    lhsT = wqt[ki][:, mi * P : (mi + 1) * P]
                    for tj in range(banks):
                        lo = tcol0 + tj * MM_N
                        nc.tensor.matmul(
                            ps[tj][:],
                            lhsT,
                            xh[ki][:, lo : lo + MM_N],
                            start=(kn == 0),
                            stop=(kn == K_TILES - 1),
                        )
            for tj in range(banks):
                ysb = y_p.tile([P, MM_N], f16, tag="ysb", name=f"y{name}_{tj}")
                if split_evict and tj % 2:
                    nc.vector.tensor_scalar(
                        out=ysb[:], in0=ps[tj][:], scalar1=s_t[:],
                        scalar2=bias_t[mi], op0=Alu.mult, op1=Alu.add,
                    )
                else:
                    nc.scalar.activation(
                        ysb[:], ps[tj][:], Act.Identity, bias=bias_t[mi],
                        scale=s_t[:],
                    )
                lo = tcol0 + tj * MM_N
                (nc.scalar if split_evict and tj % 2 else nc.sync).dma_start(
                    yt[mi * P : (mi + 1) * P, lo : lo + MM_N], ysb[:]
                )

        for half in range(2):
            for mi in range(M_TILES):
                last = half == 1 and mi == M_TILES - 1
                mm_group(
                    mi, half * HALF_TOK, TJ, f"h{half}_{mi}",
                    ki_inner=last, split_evict=last,
                )

    nc.compile()
    return nc


def _get(qmax: float):
    key = qmax
    if key not in _CACHE:
        _CACHE[key] = _build(qmax)
    return _CACHE[key]


def host_prep(x, weight):
    """Host-side input re-encoding: transpose, phase-major permute the in_f
    axis, fp16-encode x. Pure layout/encoding; no module math."""
    xt = np.ascontiguousarray(x.T)[_PERM]  # [IN_F perm, TOKENS]
    xth = xt.astype(np.float16)
    wp = np.ascontiguousarray(weight.T[_PERM])  # [IN_F perm, OUT_F]
    return xth, wp


LAST_EXEC_NS = None


def kernel(x, weight, bias, precision, _trace_dir=None):
    global LAST_EXEC_NS
    from concourse.bass_utils import run_bass_kernel_spmd

    x = np.asarray(x, dtype=np.float32)
    weight = np.asarray(weight, dtype=np.float32)
    bias = np.asarray(bias, dtype=np.float32)
    prec = int(np.asarray(precision))
    qmax = float(2 ** (prec - 1) - 1)

    nc = _get(qmax)

    xth, wp = host_prep(x, weight)
    btile = np.ascontiguousarray(bias.reshape(M_TILES, P).T)  # [128, 8]
    in_maps = [
        {
            "xth": np.ascontiguousarray(
                xth[:, c * TOK_PER_CORE : (c + 1) * TOK_PER_CORE]
            ),
            "wp": wp,
            "biast": btile,
        }
        for c in range(N_CORES)
    ]
    kw = {}
    if _trace_dir is not None:
        kw = {"trace": True, "tmpdir": _trace_dir}
    res = run_bass_kernel_spmd(nc, in_maps, list(range(N_CORES)), **kw)
    LAST_EXEC_NS = res.exec_time_ns
    yt = np.concatenate([res.results[c]["yt"] for c in range(N_CORES)], axis=1)
    return np.ascontiguousarray(yt.T.astype(np.float32))
